# revision 55
# baseline (speedup 1.0000x reference)
"""DualResolutionAttention Trainium2 kernel (8 NeuronCores, Bass/Tile).

Sharding: core c -> (batch b = c//4, group g = c%4).
Each core computes local heads {2g, 2g+1} and global heads {2g, 2g+1} over the
full sequence, plus the output channel slice [128g, 128g+128) of each branch.
Three AllGathers within each 4-core batch group: (1) compressed stream cgT,
(2) normalized local attention (fp16), (3) normalized global attention (fp16).

v2 design (vs baseline): fp16 compute everywhere (FWL weight loads, 2x less
DMA/SBUF), V computed directly in [token, vdim] layout (no PE transposes),
q/k evicted straight from PSUM to qT/kT (packed per-head weights), gate logits
folded into the projection matmuls as a 129th output column, masks via 64-row
identity matmuls (no PE tiling-mode switch inside attention).
"""
import os
import sys

sys.path.insert(0, "/opt/trn_rl_repo")
os.environ.setdefault("JAX_PLATFORMS", "axon,cpu")

from contextlib import ExitStack

import numpy as np

import concourse.bass as bass
import concourse.mybir as mybir
import concourse.tile as tile
from concourse import bacc
from concourse.bass_utils import run_bass_kernel_spmd
from concourse.masks import make_identity

FP32 = mybir.dt.float32
FP16 = mybir.dt.float16
AF = mybir.ActivationFunctionType

# Problem constants
B, T, E = 2, 4096, 1024
LD = 512            # local/global stream dim
D = 64              # head dim
HH = 8              # heads per branch
R = 4               # compression ratio
Tc = T // R         # 1024
NCORES = 8
GROUPS = [[0, 1, 2, 3], [4, 5, 6, 7]]

NEG = -30000.0      # fp16-safe mask value


# ---------------------------------------------------------------------------
# Program builder
# ---------------------------------------------------------------------------

def _attention(nc, tc, ctx, name, nQT2, qT, kTh, v_sb, comb, rec, consts,
               ones2, contribs, gathereds, anp):
    """Attention body: S^T layout scores, [v|ones] PV with denominator row.

    All matmuls run in 128-row tiling mode (kTh[h] is the per-head key tile
    with the other head's partition half zeroed), so the PE never pays a
    tiling-mode-switch drain.  The kb loop runs per 512-query half with a
    double-buffered [128, 1024] score PSUM tile (both heads side by side,
    one Exp per iteration), so the scores->exp WAR never stalls the PE.

    comb[h] is a [65, nQT2*1024] fp16 tile: rows 0:64 = unnormalized attT,
    row 64 = softmax denominator.  rec[h] [1, ncols] gets 1/denominator
    (computed in a [128, 8] layout via DMA reshape - reciprocal is
    8 cyc/elem on the DVE).  Each q2 chunk is normalized and AllGathered
    separately (contribs[q2] -> gathereds[q2]) so the collectives overlap
    later compute; the normalize matmuls for chunk q2 are deferred and
    emitted a few kb into chunk q2+1 (the PE never waits on the reciprocal
    chain).  Returns the last chunk's un-emitted normalize closure - the
    caller must invoke it after emitting some independent PE work.
    """
    ps_s = ctx.enter_context(tc.tile_pool(name=f"{name}_ps_s", bufs=2, space="PSUM"))
    ps_o = ctx.enter_context(tc.tile_pool(name=f"{name}_ps_o", bufs=1, space="PSUM"))
    p_pool = ctx.enter_context(tc.tile_pool(name=f"{name}_p", bufs=4))
    dnp = ctx.enter_context(tc.tile_pool(name=f"{name}_dn", bufs=4))

    mask_tri = consts["mask_tri"]
    ident = consts["ident"]

    def emit_scores(q2, qs, kb):
        t0 = max(0, 128 * kb - 1024 * q2 - qs)   # mask start within the half
        has_mask = 128 * kb >= 1024 * q2 + qs
        ps2 = ps_s.tile([128, 1024], FP32, name=f"{name}_s2", tag=f"{name}_s2")
        p_sb = p_pool.tile([128, 1024], FP16, name=f"{name}_pt")
        for h in range(2):
            # columns below t0 are fully masked - skip computing them
            nc.tensor.matmul(
                ps2[:, 512 * h + t0:512 * h + 512],
                kTh[h][:, 128 * kb:128 * kb + 128],
                qT[:, 1024 * q2 + qs + t0:1024 * q2 + qs + 512],
                start=True, stop=True,
            )
        if has_mask:
            for h in range(2):
                nc.tensor.matmul(
                    ps2[:, 512 * h + t0:512 * h + t0 + 128],
                    ident[:], mask_tri[:],
                    start=False, stop=True, skip_group_check=True,
                )
        if not has_mask and kb % 3 == 2:
            # Schraudolph fast exp on the (otherwise idle) DVE: write the
            # fp16 bit pattern of e^x as an int16 affine transform.
            # bits = x * 2^10/ln2 + (15*2^10 - 61); max rel err ~4%.
            with nc.allow_low_precision(reason="schraudolph exp"):
                nc.vector.tensor_scalar(
                    p_sb.bitcast(mybir.dt.int16)[:], ps2[:],
                    1477.3197, 15299.0,
                    mybir.AluOpType.mult, mybir.AluOpType.add)
        elif t0 == 0:
            nc.scalar.activation(p_sb[:], ps2[:], AF.Exp)
        else:
            p3 = p_sb.rearrange("p (h c) -> p h c", h=2, c=512)
            s3 = ps2.rearrange("p (h c) -> p h c", h=2, c=512)
            nc.scalar.activation(p3[:, :, t0:512], s3[:, :, t0:512], AF.Exp)
        return p_sb, t0

    def emit_pv(h, kb, nkb_h, qs, psum_o, p_sb, t0):
        nc.tensor.matmul(
            psum_o[:, qs + t0:qs + 512],
            v_sb[kb][:, 65 * h:65 * h + 65],
            p_sb[:, 512 * h + t0:512 * h + 512],
            start=(kb == 0), stop=(kb == nkb_h - 1),
            skip_group_check=True,
        )

    def make_norm(q2):
        def flush(pool=None, tag=None):
            pool = pool if pool is not None else ps_s
            tag = tag if tag is not None else f"{name}_s2"
            contrib, gathered = contribs[q2], gathereds[q2]
            for h in range(2):
                attn = anp.tile([64, 1024], FP16, name=f"{name}_attn")
                for c2 in range(2):
                    ps = pool.tile([128, 512], FP32, name=f"{name}_bc",
                                   tag=tag)
                    nc.tensor.matmul(
                        ps[:], ones2[0:1, :],
                        rec[h][0:1, 1024 * q2 + 512 * c2:
                               1024 * q2 + 512 * c2 + 512],
                        start=True, stop=True)
                    with nc.allow_low_precision(reason="attnorm fp16"):
                        nc.vector.tensor_mul(
                            attn[:, 512 * c2:512 * c2 + 512],
                            comb[h][0:64, 1024 * q2 + 512 * c2:
                                    1024 * q2 + 512 * c2 + 512],
                            ps[0:64, :])
                nc.sync.dma_start(out=contrib[64 * h:64 * h + 64, :],
                                  in_=attn[:])
            nc.gpsimd.collective_compute(
                "AllGather", mybir.AluOpType.bypass, replica_groups=GROUPS,
                ins=[contrib.opt()], outs=[gathered.opt()],
            )
        return flush

    pending = None
    for q2 in range(nQT2):
        psum_o = [ps_o.tile([65, 1024], FP32, name=f"{name}_o{h}")
                  for h in range(2)]
        for half in range(2):
            qs = 512 * half
            nkb_h = 8 * q2 + 4 * (half + 1)
            pend = None
            for kb in range(nkb_h):
                cur = emit_scores(q2, qs, kb)
                if pending is not None and kb == 4:
                    pending()
                    pending = None
                if pend is not None:
                    p_sb, t0 = pend
                    for h in range(2):
                        emit_pv(h, kb - 1, nkb_h, qs, psum_o[h], p_sb, t0)
                pend = cur
            p_sb, t0 = pend
            for h in range(2):
                emit_pv(h, nkb_h - 1, nkb_h, qs, psum_o[h], p_sb, t0)
        for h in range(2):
            # evict attT rows + denominator row; the two heads go to
            # different engines so the evictions run in parallel
            with nc.allow_low_precision(reason="att fp16"):
                if h == 0:
                    nc.vector.tensor_copy(
                        comb[h][:, 1024 * q2:1024 * q2 + 1024], psum_o[h][:])
                else:
                    nc.scalar.activation(
                        comb[h][:, 1024 * q2:1024 * q2 + 1024], psum_o[h][:],
                        AF.Copy)
            # reciprocal in [128, 8] layout (DMA reshape there and back);
            # the whole chain stays on the DVE queue - no sync-FIFO blocking
            dh = dnp.tile([128, 8], FP16, name=f"{name}_dh")
            nc.scalar.dma_start(
                out=dh[:], in_=comb[h][64:65, 1024 * q2:1024 * q2 + 1024])
            rc = dnp.tile([128, 8], FP16, name=f"{name}_rc")
            with nc.allow_low_precision(reason="softmax denom fp16"):
                nc.vector.reciprocal(rc[:], dh[:])
            nc.scalar.dma_start(
                out=rec[h][0:1, 1024 * q2:1024 * q2 + 1024], in_=rc[:])
        pending = make_norm(q2)
    return pending


def build_program():
    nc = bacc.Bacc(None, target_bir_lowering=False)

    def inp(name, shape, dt=FP16):
        return nc.declare_dram_parameter(name, list(shape), dt, isOutput=False)

    # data
    xlt = inp("xlt", [4, 128, T])            # x[b,:,:512].T chunks
    xct = inp("xct", [32, 128, Tc])          # x[b].reshape(Tc,4096).T chunks
    # weights
    wq = inp("wq", [4, 128, 128])            # [qA|qB] lhsT chunks (scaled 1/8)
    bq = inp("bq", [128, 1], FP32)
    wk = inp("wk", [4, 128, 128])
    bk = inp("bk", [128, 1], FP32)
    wv = inp("wv", [4, 128, 128])            # [vA|vB] (rhs for v-direct)
    bvb = inp("bvb", [128, 128])             # [bvA|bvB] replicated to 128 rows
    wgq = inp("wgq", [4, 128, 128])
    bgq = inp("bgq", [128, 1], FP32)
    wgk = inp("wgk", [4, 128, 128])
    bgk = inp("bgk", [128, 1], FP32)
    wgv = inp("wgv", [4, 128, 128])
    bgvb = inp("bgvb", [128, 128])
    wc = inp("wc", [32, 128, 128])           # compress slice lhsT chunks
    bc = inp("bc", [128, 1], FP32)
    wplz = inp("wplz", [4, 128, 129])        # [w_lproj[:,cs] | u_l] chunks
    bplzb = inp("bplzb", [128, 129])         # [b_lproj[cs] | c0] replicated
    wpgz = inp("wpgz", [4, 128, 129])
    bpgzb = inp("bpgzb", [128, 129])
    repA = inp("repA", [128, 128])           # x4 expander (even 32-blocks)
    repB = inp("repB", [128, 128])           # x4 expander (odd 32-blocks)
    maskt = inp("maskt", [128, 128])         # strict lower-tri NEG
    out_loc = nc.declare_dram_parameter("out_loc", [T, 128], FP16, isOutput=True)
    out_glob = nc.declare_dram_parameter("out_glob", [T, 128], FP16, isOutput=True)

    with tile.TileContext(nc) as tc:
      with ExitStack() as top:
        dram = top.enter_context(tc.tile_pool(name="dram", bufs=1, space="DRAM"))
        const = top.enter_context(tc.tile_pool(name="const", bufs=1))
        persist = top.enter_context(tc.tile_pool(name="persist", bufs=1))

        # constants (DMA issues for these are deferred into phase B so the
        # first qkv matmul's inputs go out on the queue first)
        ident = const.tile([128, 128], FP16, name="ident")
        make_identity(nc, ident[:])
        mask_tri = const.tile([128, 128], FP16, name="mask_tri")
        repA_sb = const.tile([128, 128], FP16, name="repA_sb")
        repB_sb = const.tile([128, 128], FP16, name="repB_sb")
        ones2 = const.tile([1, 128], FP16, name="ones2")
        nc.gpsimd.memset(ones2[:], 1.0)
        consts = {"mask_tri": mask_tri, "ident": ident}
        biases = {nm: const.tile([128, 1], FP32, name=f"cb_{nm}")
                  for nm in ("bq", "bk", "bgq", "bgk", "bc")}
        brows = {nm: const.tile([128, w], FP16, name=f"br_{nm}")
                 for nm, w in (("bvb", 128), ("bgvb", 128),
                               ("bplzb", 129), ("bpgzb", 129))}

        def load_consts():
            nc.sync.dma_start(out=biases["bq"][:], in_=bq[:])
            nc.sync.dma_start(out=biases["bk"][:], in_=bk[:])
            nc.sync.dma_start(out=mask_tri[:], in_=maskt[:])
            nc.sync.dma_start(out=brows["bvb"][:], in_=bvb[:])
            for t, src in ((biases["bgq"], bgq), (biases["bgk"], bgk),
                           (biases["bc"], bc), (brows["bgvb"], bgvb),
                           (brows["bplzb"], bplzb), (brows["bpgzb"], bpgzb)):
                nc.sync.dma_start(out=t[:], in_=src[:])
            nc.sync.dma_start(out=repA_sb[:], in_=repA[:])
            nc.sync.dma_start(out=repB_sb[:], in_=repB[:])

        # persistent attention inputs.  kT is stored per head with the other
        # head's partition half zeroed so score matmuls run at K=128 (no PE
        # tiling-mode switches, FWL-eligible weight loads).
        qT_l = persist.tile([128, T], FP16, name="qT_l")
        kT_lh = [persist.tile([128, T], FP16, name=f"kT_l{h}") for h in range(2)]
        qT_g = persist.tile([128, Tc], FP16, name="qT_g")
        kT_gh = [persist.tile([128, Tc], FP16, name=f"kT_g{h}") for h in range(2)]
        nc.gpsimd.memset(kT_lh[0][64:128, :], 0.0)
        nc.gpsimd.memset(kT_lh[1][0:64, :], 0.0)
        nc.gpsimd.memset(kT_gh[0][64:128, :], 0.0)
        nc.gpsimd.memset(kT_gh[1][0:64, :], 0.0)
        v_sb_l = [persist.tile([128, 130], FP16, name=f"vsb{i}")
                  for i in range(32)]
        v_sb_g = [persist.tile([128, 130], FP16, name=f"vgsb{i}")
                  for i in range(8)]
        cg_all = [persist.tile([128, Tc], FP16, name=f"cg_all{i}")
                  for i in range(4)]
        # ones columns for the PV denominator row (cols 64 and 129)
        for v_tiles in (v_sb_l, v_sb_g):
            for vt in v_tiles:
                nc.vector.memset(vt[:, 64:65], 1.0)
                nc.vector.memset(vt[:, 129:130], 1.0)

        # ------------------------------------------------------ phase B: local qkv
        pab = top.enter_context(ExitStack())
        with ExitStack() as pb:
            xlp = pb.enter_context(tc.tile_pool(name="xlt_pool", bufs=1))
            wqp = pb.enter_context(tc.tile_pool(name="wq_pool", bufs=1))
            psB = pb.enter_context(tc.tile_pool(name="psB", bufs=3, space="PSUM"))
            psV = pb.enter_context(tc.tile_pool(name="psV", bufs=2, space="PSUM"))

            # weights first (tiny transfers), then the big x stream: the
            # first qkv matmul only waits on wq + xlt[0]
            wq_sb, wk_sb, wv_sb = [], [], []
            for ch in range(4):
                for (lst, src, nm) in ((wq_sb, wq, "wq"), (wk_sb, wk, "wk"),
                                       (wv_sb, wv, "wv")):
                    t = wqp.tile([128, 128], FP16, name=f"{nm}{ch}")
                    nc.sync.dma_start(out=t[:], in_=src[ch])
                    lst.append(t)
            xlt_sb = []
            for ch in range(4):
                xt = xlp.tile([128, T], FP16, name=f"xlt{ch}")
                nc.sync.dma_start(out=xt[:, 0:2048], in_=xlt[ch][:, 0:2048])
                xlt_sb.append(xt)
            for ch in range(4):
                nc.sync.dma_start(out=xlt_sb[ch][:, 2048:T],
                                  in_=xlt[ch][:, 2048:T])
            load_consts()

            # q and k: packed [headA|headB] out dims -> direct eviction
            kT_st = xlp.tile([128, T], FP16, name="kT_st")
            for (wsb, dst, bias_ap) in ((wq_sb, qT_l, biases["bq"][:]),
                                        (wk_sb, kT_st, biases["bk"][:])):
                for qt in range(8):
                    ps = psB.tile([128, 512], FP32, name="psB_t")
                    for ch in range(4):
                        nc.tensor.matmul(
                            ps[:], wsb[ch][:],
                            xlt_sb[ch][:, 512 * qt:512 * qt + 512],
                            start=(ch == 0), stop=(ch == 3))
                    with nc.allow_low_precision(reason="qk fp16"):
                        nc.scalar.activation(dst[:, 512 * qt:512 * qt + 512],
                                             ps[:], AF.Identity, bias=bias_ap)
            # split k into per-head zero-padded tiles (partition-preserving)
            nc.sync.dma_start(out=kT_lh[0][0:64, :], in_=kT_st[0:64, :])
            nc.sync.dma_start(out=kT_lh[1][64:128, :], in_=kT_st[64:128, :])

            # v: direct [token, vdim] layout, bias added at eviction
            bvb3 = brows["bvb"].rearrange("p (h c) -> p h c", h=2, c=64)
            for tb in range(32):
                ps = psV.tile([128, 128], FP32, name="psV_t")
                for ch in range(4):
                    nc.tensor.matmul(
                        ps[:], xlt_sb[ch][:, 128 * tb:128 * tb + 128],
                        wv_sb[ch][:], start=(ch == 0), stop=(ch == 3))
                v3 = v_sb_l[tb].rearrange("p (h c) -> p h c", h=2, c=65)
                p3 = ps.rearrange("p (h c) -> p h c", h=2, c=64)
                with nc.allow_low_precision(reason="v fp16"):
                    nc.vector.tensor_add(v3[:, :, 0:64], p3[:], bvb3[:])

        # ------------------------------------------------------ phase A: compress
        xp = pab.enter_context(tc.tile_pool(name="xct_pool", bufs=8))
        wp = pab.enter_context(tc.tile_pool(name="wc_pool", bufs=8))
        cgp = pab.enter_context(tc.tile_pool(name="cg_pool", bufs=1))
        psA = pab.enter_context(tc.tile_pool(name="psA", bufs=1, space="PSUM"))
        cgT = cgp.tile([128, Tc], FP16, name="cgT")
        ps0 = psA.tile([128, 512], FP32, name="psA_0")
        ps1 = psA.tile([128, 512], FP32, name="psA_1")
        for ch in range(32):
            wt = wp.tile([128, 128], FP16, name="wc_t")
            nc.sync.dma_start(out=wt[:], in_=wc[ch])
            xt = xp.tile([128, Tc], FP16, name="xct_t")
            nc.sync.dma_start(out=xt[:], in_=xct[ch])
            nc.tensor.matmul(ps0[:], wt[:], xt[:, 0:512],
                             start=(ch == 0), stop=(ch == 31))
            nc.tensor.matmul(ps1[:], wt[:], xt[:, 512:1024],
                             start=(ch == 0), stop=(ch == 31))
        with nc.allow_low_precision(reason="cg fp16"):
            nc.scalar.activation(cgT[:, 0:512], ps0[:],
                                 AF.Identity, bias=biases["bc"][:])
            nc.scalar.activation(cgT[:, 512:1024], ps1[:],
                                 AF.Identity, bias=biases["bc"][:])
        cg_contrib = dram.tile([128, Tc], FP16, name="cg_contrib")
        cg_gathered = dram.tile([512, Tc], FP16, name="cg_gathered")
        nc.sync.dma_start(out=cg_contrib[:], in_=cgT[:])
        nc.gpsimd.collective_compute(
            "AllGather", mybir.AluOpType.bypass, replica_groups=GROUPS,
            ins=[cg_contrib.opt()], outs=[cg_gathered.opt()],
        )
        for i in range(4):
            nc.sync.dma_start(out=cg_all[i][:],
                              in_=cg_gathered[128 * i:128 * i + 128, :])
        pab.close()

        # ------------------------------------------------------ phase C: attention
        cpool = top.enter_context(tc.tile_pool(name="c_pool", bufs=1))
        comb_l = [cpool.tile([65, T], FP16, name=f"comb_l{h}") for h in range(2)]
        comb_g = [cpool.tile([65, Tc], FP16, name=f"comb_g{h}") for h in range(2)]
        rec_l = [cpool.tile([1, T], FP16, name=f"rec_l{h}") for h in range(2)]
        rec_g = [cpool.tile([1, Tc], FP16, name=f"rec_g{h}") for h in range(2)]

        contribs_l = [dram.tile([128, Tc], FP16, name=f"attnl_c{i}")
                      for i in range(4)]
        gathereds_l = [dram.tile([512, Tc], FP16, name=f"attnl_g{i}")
                       for i in range(4)]
        contrib_g = dram.tile([128, Tc], FP16, name="attng_contrib")
        gathered_g = dram.tile([512, Tc], FP16, name="attng_gathered")

        app = top.enter_context(tc.tile_pool(name="attall_pool", bufs=1))
        att_all = [app.tile([128, T], FP16, name=f"attall{i}") for i in range(4)]
        anp_top = top.enter_context(tc.tile_pool(name="anp_top", bufs=2))

        pc1 = top.enter_context(ExitStack())
        pend_l = _attention(nc, tc, pc1, "la", 4, qT_l, kT_lh, v_sb_l, comb_l,
                            rec_l, consts, ones2, contribs_l, gathereds_l,
                            anp_top)
        pc1.close()
        # att_all chunk DMAs for the already-gathered q2 chunks
        for q2 in range(3):
            for i in range(4):
                nc.gpsimd.dma_start(
                    out=att_all[i][:, 1024 * q2:1024 * q2 + 1024],
                    in_=gathereds_l[q2][128 * i:128 * i + 128, :])

        # global qkv emitted before the last local normalize chunk so the PE
        # stream never stalls on the reciprocal/bcast chain
        with ExitStack() as pg:
            wgp = pg.enter_context(tc.tile_pool(name="wg_pool", bufs=1))
            psG = pg.enter_context(tc.tile_pool(name="psG", bufs=3, space="PSUM"))
            psGV = pg.enter_context(tc.tile_pool(name="psGV", bufs=2, space="PSUM"))
            wgq_sb, wgk_sb, wgv_sb = [], [], []
            for ch in range(4):
                for (lst, src, nm) in ((wgq_sb, wgq, "wgq"), (wgk_sb, wgk, "wgk"),
                                       (wgv_sb, wgv, "wgv")):
                    t = wgp.tile([128, 128], FP16, name=f"{nm}{ch}")
                    nc.sync.dma_start(out=t[:], in_=src[ch])
                    lst.append(t)
            kTg_st = wgp.tile([128, Tc], FP16, name="kTg_st")
            for (wsb, dst, bias_ap) in ((wgq_sb, qT_g, biases["bgq"][:]),
                                        (wgk_sb, kTg_st, biases["bgk"][:])):
                for qt in range(2):
                    ps = psG.tile([128, 512], FP32, name="psG_t")
                    for ch in range(4):
                        nc.tensor.matmul(
                            ps[:], wsb[ch][:],
                            cg_all[ch][:, 512 * qt:512 * qt + 512],
                            start=(ch == 0), stop=(ch == 3))
                    with nc.allow_low_precision(reason="gqk fp16"):
                        nc.scalar.activation(dst[:, 512 * qt:512 * qt + 512],
                                             ps[:], AF.Identity, bias=bias_ap)
            nc.sync.dma_start(out=kT_gh[0][0:64, :], in_=kTg_st[0:64, :])
            nc.sync.dma_start(out=kT_gh[1][64:128, :], in_=kTg_st[64:128, :])
            # flush the last local normalize chunk now that the PE has
            # independent global-qkv work queued ahead of it
            pend_l(psG, "psG_t")
            for i in range(4):
                nc.gpsimd.dma_start(
                    out=att_all[i][:, 3072:4096],
                    in_=gathereds_l[3][128 * i:128 * i + 128, :])
            bgvb3 = brows["bgvb"].rearrange("p (h c) -> p h c", h=2, c=64)
            for tb in range(8):
                ps = psGV.tile([128, 128], FP32, name="psGV_t")
                for ch in range(4):
                    nc.tensor.matmul(
                        ps[:], cg_all[ch][:, 128 * tb:128 * tb + 128],
                        wgv_sb[ch][:], start=(ch == 0), stop=(ch == 3))
                v3 = v_sb_g[tb].rearrange("p (h c) -> p h c", h=2, c=65)
                p3 = ps.rearrange("p (h c) -> p h c", h=2, c=64)
                with nc.allow_low_precision(reason="gv fp16"):
                    nc.vector.tensor_add(v3[:, :, 0:64], p3[:], bgvb3[:])

        pc2 = top.enter_context(ExitStack())
        pend_g = _attention(nc, tc, pc2, "ga", 1, qT_g, kT_gh, v_sb_g, comb_g,
                            rec_g, consts, ones2, [contrib_g], [gathered_g],
                            anp_top)
        pc2.close()

        # ------------------------------------------------------ phase D: proj+gate
        with ExitStack() as pd:
            wpp = pd.enter_context(tc.tile_pool(name="wp_pool", bufs=1))
            psP = pd.enter_context(tc.tile_pool(name="psP", bufs=3, space="PSUM"))
            psE = pd.enter_context(tc.tile_pool(name="psE", bufs=2, space="PSUM"))
            gp = pd.enter_context(tc.tile_pool(name="gproj_pool", bufs=1))
            zp = pd.enter_context(tc.tile_pool(name="z_pool", bufs=1))
            outp = pd.enter_context(tc.tile_pool(name="out_pool", bufs=4))

            wplz_sb, wpgz_sb = [], []
            for ch in range(4):
                t = wpp.tile([128, 129], FP16, name=f"wplz{ch}")
                nc.sync.dma_start(out=t[:], in_=wplz[ch])
                wplz_sb.append(t)
                t = wpp.tile([128, 129], FP16, name=f"wpgz{ch}")
                nc.sync.dma_start(out=t[:], in_=wpgz[ch])
                wpgz_sb.append(t)

            # global normalize first so the attn_g gather starts ASAP and
            # overlaps the local proj matmuls
            pend_g(psP, "psP_t")

            # local proj: [128 tok, 129] blocks -> loc_sb
            loc_sb = gp.tile([128, 32 * 129], FP16, name="loc_sb")
            loc3 = loc_sb.rearrange("p (a b) -> p a b", a=32, b=129)
            for tb in range(32):
                ps = psP.tile([128, 129], FP32, name="psP_t")
                for ch in range(4):
                    nc.tensor.matmul(ps[:],
                                     att_all[ch][:, 128 * tb:128 * tb + 128],
                                     wplz_sb[ch][:], start=(ch == 0), stop=(ch == 3))
                with nc.allow_low_precision(reason="proj fp16"):
                    nc.vector.tensor_add(loc3[:, tb, :], ps[:], brows["bplzb"][:])
            attg_all = []
            for i in range(4):
                t = wpp.tile([128, Tc], FP16, name=f"attgall{i}")
                nc.gpsimd.dma_start(out=t[:],
                                    in_=gathered_g[128 * i:128 * i + 128, :])
                attg_all.append(t)

            # global proj (Tc rows) -> gproj_sb, then x4 expand -> ge_sb
            gproj_sb = gp.tile([128, 8 * 129], FP16, name="gproj_sb")
            gproj3 = gproj_sb.rearrange("p (a b) -> p a b", a=8, b=129)
            for tbg in range(8):
                ps = psP.tile([128, 129], FP32, name="psP_t")
                for ch in range(4):
                    nc.tensor.matmul(ps[:],
                                     attg_all[ch][:, 128 * tbg:128 * tbg + 128],
                                     wpgz_sb[ch][:], start=(ch == 0), stop=(ch == 3))
                with nc.allow_low_precision(reason="gproj fp16"):
                    nc.vector.tensor_add(gproj3[:, tbg, :], ps[:], brows["bpgzb"][:])
            ge_sb = gp.tile([128, 32 * 129], FP16, name="ge_sb")
            ge3 = ge_sb.rearrange("p (a b) -> p a b", a=32, b=129)
            for tb in range(32):
                base = 64 * ((tb % 4) // 2)
                rep = repA_sb if tb % 2 == 0 else repB_sb
                ps = psE.tile([128, 129], FP32, name="psE_t")
                nc.tensor.matmul(ps[:], rep[base:base + 64, :],
                                 gproj3[base:base + 64, tb // 4, :],
                                 start=True, stop=True)
                with nc.allow_low_precision(reason="gexp fp16"):
                    nc.vector.tensor_copy(ge3[:, tb, :], ps[:])

            # gate in groups of 8 blocks: z = loc_z + ge_z;
            # g0 = 0.5 + 0.5*tanh(z/2); g1 = 1 - g0; batched out DMAs
            for grp in range(4):
                b0 = 8 * grp
                zsum = zp.tile([128, 8], FP32, name="zsum")
                nc.vector.tensor_add(zsum[:], loc3[:, b0:b0 + 8, 128],
                                     ge3[:, b0:b0 + 8, 128])
                tanh_t = zp.tile([128, 8], FP32, name="tanh_t")
                nc.scalar.activation(tanh_t[:], zsum[:], AF.Tanh, scale=0.5)
                g0 = zp.tile([128, 8], FP32, name="g0")
                g1 = zp.tile([128, 8], FP32, name="g1")
                nc.vector.tensor_scalar(g0[:], tanh_t[:], 0.5, 0.5,
                                        mybir.AluOpType.mult,
                                        mybir.AluOpType.add)
                nc.vector.tensor_scalar(g1[:], tanh_t[:], -0.5, 0.5,
                                        mybir.AluOpType.mult,
                                        mybir.AluOpType.add)
                ol = outp.tile([128, 8 * 128], FP16, name="outl")
                ol3 = ol.rearrange("p (b c) -> p b c", b=8, c=128)
                og = outp.tile([128, 8 * 128], FP16, name="outg")
                og3 = og.rearrange("p (b c) -> p b c", b=8, c=128)
                for j in range(8):
                    tb = b0 + j
                    with nc.allow_low_precision(reason="out fp16"):
                        nc.vector.tensor_scalar_mul(ol3[:, j, :],
                                                    loc3[:, tb, 0:128],
                                                    g0[:, j:j + 1])
                        nc.vector.tensor_scalar_mul(og3[:, j, :],
                                                    ge3[:, tb, 0:128],
                                                    g1[:, j:j + 1])
                out_l_v = out_loc[1024 * grp:1024 * grp + 1024, :] \
                    .rearrange("(b p) c -> p b c", b=8, p=128)
                nc.sync.dma_start(out=out_l_v, in_=ol3[:])
                out_g_v = out_glob[1024 * grp:1024 * grp + 1024, :] \
                    .rearrange("(b p) c -> p b c", b=8, p=128)
                nc.sync.dma_start(out=out_g_v, in_=og3[:])

    nc.finalize()
    return nc


# ---------------------------------------------------------------------------
# Host side
# ---------------------------------------------------------------------------

_NC_CACHE = []


def _get_program():
    if not _NC_CACHE:
        _NC_CACHE.append(build_program())
    return _NC_CACHE[0]


def _prep_inputs(x, w_lqkv, b_lqkv, w_gqkv, b_gqkv, w_comp, b_comp,
                 w_lproj, b_lproj, w_gproj, b_gproj, w_gate, b_gate):
    f32, f16 = np.float32, np.float16
    wd = (w_gate[:, 0] - w_gate[:, 1]).astype(f32)
    u_l = (w_lproj @ wd[:LD]).astype(f32)
    u_g = (w_gproj @ wd[LD:]).astype(f32)
    c0 = float(b_lproj @ wd[:LD] + b_gproj @ wd[LD:] + b_gate[0] - b_gate[1])

    mask_tri = np.where(np.arange(128)[None, :] >= np.arange(128)[:, None],
                        0.0, NEG).astype(f16)
    e0 = np.zeros((64, 128), f32)
    e0[np.arange(128) // 4, np.arange(128)] = 1.0
    e1 = np.zeros((64, 128), f32)
    e1[32 + np.arange(128) // 4, np.arange(128)] = 1.0
    repA_ = np.concatenate([e0, e0], axis=0).astype(f16)
    repB_ = np.concatenate([e1, e1], axis=0).astype(f16)

    def packed_cols(w, b, off, ha, hb, scale=1.0):
        wp = np.concatenate([w[:, off + D * ha:off + D * ha + D],
                             w[:, off + D * hb:off + D * hb + D]], axis=1) * scale
        bp = np.concatenate([b[off + D * ha:off + D * ha + D],
                             b[off + D * hb:off + D * hb + D]]) * scale
        return wp.astype(f16).reshape(4, 128, 128), bp.astype(f32).reshape(128, 1)

    in_maps = []
    for core in range(NCORES):
        b_idx, g = core // 4, core % 4
        ha, hb = 2 * g, 2 * g + 1
        cs = slice(128 * g, 128 * g + 128)

        xlt_ = np.ascontiguousarray(x[b_idx, :, :LD].T).astype(f16).reshape(4, 128, T)
        xct_ = np.ascontiguousarray(
            x[b_idx].reshape(Tc, R * E).T).astype(f16).reshape(32, 128, Tc)

        wq_, bq_ = packed_cols(w_lqkv, b_lqkv, 0, ha, hb, 1.0 / 8.0)
        wk_, bk_ = packed_cols(w_lqkv, b_lqkv, LD, ha, hb)
        wv_, bv_ = packed_cols(w_lqkv, b_lqkv, 2 * LD, ha, hb)
        wgq_, bgq_ = packed_cols(w_gqkv, b_gqkv, 0, ha, hb, 1.0 / 8.0)
        wgk_, bgk_ = packed_cols(w_gqkv, b_gqkv, LD, ha, hb)
        wgv_, bgv_ = packed_cols(w_gqkv, b_gqkv, 2 * LD, ha, hb)

        wplz_ = np.concatenate(
            [w_lproj[:, cs], u_l[:, None]], axis=1).astype(f16).reshape(4, 128, 129)
        wpgz_ = np.concatenate(
            [w_gproj[:, cs], u_g[:, None]], axis=1).astype(f16).reshape(4, 128, 129)

        in_maps.append({
            "xlt": xlt_, "xct": xct_,
            "wq": wq_, "bq": bq_, "wk": wk_, "bk": bk_,
            "wv": wv_,
            "bvb": np.tile(bv_.reshape(1, 128), (128, 1)).astype(f16),
            "wgq": wgq_, "bgq": bgq_, "wgk": wgk_, "bgk": bgk_,
            "wgv": wgv_,
            "bgvb": np.tile(bgv_.reshape(1, 128), (128, 1)).astype(f16),
            "wc": np.ascontiguousarray(
                w_comp[:, LD + 128 * g:LD + 128 * g + 128]).astype(f16)
                .reshape(32, 128, 128),
            "bc": b_comp[LD + 128 * g:LD + 128 * g + 128].astype(f32)
                .reshape(128, 1),
            "wplz": wplz_,
            "bplzb": np.tile(np.concatenate([b_lproj[cs], [c0]])
                             .reshape(1, 129), (128, 1)).astype(f16),
            "wpgz": wpgz_,
            "bpgzb": np.tile(np.concatenate([b_gproj[cs], [0.0]])
                             .reshape(1, 129), (128, 1)).astype(f16),
            "repA": repA_, "repB": repB_, "maskt": mask_tri,
        })
    return in_maps


def _run(in_maps, trace=False):
    nc = _get_program()
    return run_bass_kernel_spmd(nc, in_maps, list(range(NCORES)), trace=trace)


def assemble(results):
    out = np.empty((B, T, E), np.float32)
    for core in range(NCORES):
        b_idx, g = core // 4, core % 4
        out[b_idx, :, 128 * g:128 * g + 128] = \
            results[core]["out_loc"].astype(np.float32)
        out[b_idx, :, LD + 128 * g:LD + 128 * g + 128] = \
            results[core]["out_glob"].astype(np.float32)
    return out


def kernel(**inputs):
    in_maps = _prep_inputs(**inputs)
    res = _run(in_maps)
    return assemble(res.results)


def kernel_traced(**inputs):
    """test.py helper: returns (output, BassKernelResults with timing)."""
    in_maps = _prep_inputs(**inputs)
    res = _run(in_maps, trace=True)
    return assemble(res.results), res
